# revision 60
# baseline (speedup 1.0000x reference)
"""DARNN (dual-attention RNN) Trainium2 Bass kernel — v5.

Strategy (pure data parallel, 8 cores, B=1024 -> 128 samples/core):

Activations are feature-major on-chip: x[b, f] lives in SBUF as xT[f, b]
(features on partitions, local batch on the free dim). Matmuls contract
over partitions with pre-transposed bf16 weights stationary.

v2 foundation:
- Decoder softmax is batch-major: attn2 uses the (feature-major) tanh
  output as the *stationary* operand, producing scores [b, 257] in one
  PSUM bank (bias folded in via a ones-row matmul). A single Exp
  activation with accum_out yields e and Z; xi returns to row layout
  via one PE transpose.  (NOTE: the fused DVE tensor_tensor_reduce
  hangs the device — NRT_EXEC_UNIT_UNRECOVERABLE — use separate
  tensor_mul + tensor_reduce.)
- All decoder sigmoids are rewritten as 0.5*tanh(x/2)+0.5 with the 0.5/2
  factors folded into packed weights (h'=2h, c'=2c representation), so
  the decoder only ever uses {Tanh, Exp} -> zero ACT table reloads.
- Decoder gate biases (+ comb_fc bias) are folded into the [xi; 1]
  matmul chunk; attn1 feat contribution for t+1 prefilled during t.

v3-v5 changes (trace-driven):
- ps_s PSUM pool bufs=2: the attn2-bias matmul no longer WAR-stalls at
  the head of the PE FIFO (~2.3us/step).
- Decoder PE issue order: attn2 placed right after the aTd tanh with
  only 4 gates-h matmuls covering the tanh latency (was: all 16 ahead
  of it); remaining gates-h + prefill fill the softmax window.
- attn1 h/c accumulation issues the c-dependent matmuls first: cd_b is
  ready ~1us before hdT, so they break up the LSTM-tail PE-idle gap.
- gates-[xi;1] stationary zero-padded from K=2 to K=128 -> FWL applies.
- Decoder gate tanh split into [i,f] / [g,o] halves so the DVE t2 op
  overlaps the second half.
- HAM warmth economics (measured): the PE un-throttles (1.2->2.4 GHz)
  only when the activity window stays densely busy, so junk "filler"
  matmuls PAY: removing them makes the whole kernel ~15% slower.
  Decoder fillers are data-anchored on th8/cd_b so the scheduler cannot
  hoist them out of the tail, and a final PSUM->DRAM dump ("fldump")
  keeps them live.  Encoder keeps the v2 5x512 tail fillers.
- Run-to-run HW variance is ~0.5-0.9ms (device throttle-state drift);
  measure twice before believing any scheduling delta.
"""

import numpy as np
import ml_dtypes

B, T, NF, HE, HD = 1024, 256, 128, 256, 256
NCORES = 8
BL = B // NCORES  # 128 local batch
TP = T + 1  # 257
BF16 = ml_dtypes.bfloat16

_CACHE = {}


def _bf(x):
    return np.ascontiguousarray(np.asarray(x).astype(BF16))


def _f32(x):
    return np.ascontiguousarray(np.asarray(x).astype(np.float32))


def _pack_inputs(inputs):
    """Pack weights/biases into per-SBUF-tile layouts (shared across cores)."""
    f = {k: np.asarray(v, dtype=np.float32) for k, v in inputs.items()}
    p = {}

    # --- encoder attn1: a = tanh(cat[x,h,c] @ Wa1.T + ba1) ---
    # lhsT tile [128, 5, 257]: [p, k, m] = Wa1[m, k*128+p]
    wa1 = f["enc_Wa1"]  # [257, 640]
    p["w_enc_a1"] = _bf(wa1.T.reshape(5, 128, TP).transpose(1, 0, 2))
    b1 = np.zeros((1, 384), np.float32)
    b1[0, :TP] = f["enc_ba1"]  # bias row for the psum bias-matmul
    p["ba1_row"] = _bf(b1)

    # --- encoder attn2: alpha = a @ Wa2.T + ba2 ---  K=257 (3 chunks), M=128
    wa2 = f["enc_Wa2"]  # [128, 257]
    w = np.zeros((128, 3, 128), np.float32)
    w2 = wa2.T  # [257, 128]
    w[:, 0, :] = w2[0:128]
    w[:, 1, :] = w2[128:256]
    w[0, 2, :] = w2[256]
    p["w_enc_a2"] = _bf(w)
    p["b_enc_a2"] = _f32(f["enc_ba2"].reshape(128, 1))

    # --- encoder gates: g = Wih@xi + Whh@h + b ---  K chunks: [xi, h0, h1]
    # column order j = [i0,i1,f0,f1,g0,g1,o0,o1] (natural torch order)
    w = np.zeros((128, 3, 4 * HE), np.float32)
    w[:, 0, :] = f["enc_Wih"].T  # [128, 1024]
    w[:, 1, :] = f["enc_Whh"].T[0:128]
    w[:, 2, :] = f["enc_Whh"].T[128:256]
    p["w_enc_g"] = _bf(w)
    bsum = f["enc_bih"] + f["enc_bhh"]
    p["b_enc_g_row"] = _bf(bsum.reshape(1, 4 * HE))

    # --- q projections: q_c = h . Wc[0,1:], q_f = h . Wf[0,HD:] ---
    w = np.zeros((128, 2, 2), np.float32)
    w[:, 0, 0] = f["dec_Wc"][0, 1 : 1 + 128]
    w[:, 1, 0] = f["dec_Wc"][0, 129 : 1 + 256]
    w[:, 0, 1] = f["dec_Wf"][0, HD : HD + 128]
    w[:, 1, 1] = f["dec_Wf"][0, HD + 128 : HD + 256]
    p["w_q"] = _bf(w)

    # --- decoder attn1: a = tanh(cat[h,c,feat] @ Wa1.T + ba1) --- K=768 (6)
    # decoder h,c are stored as h'=2h, c'=2c -> scale those k-chunks by 0.5
    wa1d = f["dec_Wa1"].copy()  # [256, 768]
    wa1d[:, 0:512] *= 0.5  # h and c columns
    p["w_dec_a1"] = _bf(wa1d.T.reshape(6, 128, HE).transpose(1, 0, 2))
    p["dba1_row"] = _bf(f["dec_ba1"].reshape(1, HE))

    # --- decoder attn2 (batch-major): s[b, t'] = a.T @ Wa2dT + ba2 ---
    # moving operand [p=feat chunk, k, n=t'] = Wa2d[n, k*128+p]
    wa2d = f["dec_Wa2"]  # [257, 256]
    p["w_dec_a2"] = _bf(wa2d.T.reshape(2, 128, TP).transpose(1, 0, 2))
    ba2r = np.zeros((1, TP), np.float32)
    ba2r[0, :] = f["dec_ba2"]
    p["ba2_row"] = _bf(ba2r)

    # --- decoder gates ---
    # tanh-form LSTM: i,f,o gates become tanh(0.5*(pre+b)); g stays tanh.
    # Fold: h' = 2h -> Whh columns *0.5 ; g-gate pre-act scaled *2 so a
    # uniform ACT scale=0.5 works for the whole tile.
    sgate = np.ones((4 * HD,), np.float32)
    sgate[512:768] = 2.0  # g-gate columns
    whh = f["dec_Whh"].T * 0.5  # [256, 1024] (h' fold)
    w = np.zeros((128, 2, 4 * HD), np.float32)
    w[:, 0, :] = whh[0:128] * sgate
    w[:, 1, :] = whh[128:256] * sgate
    p["w_dec_g"] = _bf(w)
    # k=2 chunk [xi_nc; 1] with xi_nc = y*Wc00 + dot_c/Z  (bc folded here)
    wih = f["dec_Wih"][:, 0]  # [1024]
    bsum = f["dec_bih"] + f["dec_bhh"] + wih * f["dec_bc"][0]
    gx = np.zeros((128, 4 * HD), np.float32)  # K padded to 128 -> FWL
    gx[0, :] = wih * sgate
    gx[1, :] = bsum * sgate
    p["w_dec_gx"] = _bf(gx)

    # --- final: out = hd . Wf[0,:HD] + dot_f/Z + bf ---  (hd' = 2hd fold)
    w = np.zeros((128, 2, 1), np.float32)
    w[:, 0, 0] = f["dec_Wf"][0, 0:128] * 0.5
    w[:, 1, 0] = f["dec_Wf"][0, 128:256] * 0.5
    p["w_fh"] = _bf(w)

    # --- broadcast consts: [bc, bf, Wc00, 0] replicated over partitions ---
    cb = np.zeros((128, 4), np.float32)
    cb[:, 0] = f["dec_bc"][0]
    cb[:, 1] = f["dec_bf"][0]
    cb[:, 2] = f["dec_Wc"][0, 0]
    p["consts_b"] = _f32(cb)

    # --- identity for PE transposes ---
    p["ident"] = _bf(np.eye(128, dtype=np.float32))

    # --- xiT2 init: row 1 = ones, rest 0 (row 0 rewritten per step) ---
    xi0 = np.zeros((128, BL), np.float32)
    xi0[1, :] = 1.0
    p["xi_init"] = _bf(xi0)

    # --- f32 identity (stationary for f32-moving filler matmuls) ---
    p["ident_f32"] = _f32(np.eye(128, dtype=np.float32))

    # --- per-core tensors ---
    feat = f["feat"]  # [B, 257, 128]
    # tgw = target * Wc00, host-precomputed (b-major)
    tgw = f["target"] * f["dec_Wc"][0, 0]
    per_core = []
    for c in range(NCORES):
        sl = slice(c * BL, (c + 1) * BL)
        featT = _bf(feat[sl].transpose(2, 1, 0))  # [f=128, t=257, b=128]
        per_core.append({"featT": featT, "tgw": _f32(tgw[sl])})
    return p, per_core


def _build(enc_steps=TP, dec_steps=T):
    import concourse.mybir as mybir
    from concourse import bacc
    from concourse.tile import TileContext

    dt = mybir.dt
    AF = mybir.ActivationFunctionType
    OP = mybir.AluOpType

    nc = bacc.Bacc("TRN2")

    # ---- DRAM parameters ----
    dram = {}

    def din(name, shape, dtype):
        dram[name] = nc.declare_dram_parameter(name, list(shape), dtype, isOutput=False)

    din("featT", (128, TP, BL), dt.bfloat16)
    din("tgw", (BL, T), dt.float32)
    din("w_enc_a1", (128, 5, TP), dt.bfloat16)
    din("ba1_row", (1, 384), dt.bfloat16)
    din("w_enc_a2", (128, 3, 128), dt.bfloat16)
    din("b_enc_a2", (128, 1), dt.float32)
    din("w_enc_g", (128, 3, 4 * HE), dt.bfloat16)
    din("b_enc_g_row", (1, 4 * HE), dt.bfloat16)
    din("w_q", (128, 2, 2), dt.bfloat16)
    din("w_dec_a1", (128, 6, HE), dt.bfloat16)
    din("dba1_row", (1, HE), dt.bfloat16)
    din("w_dec_a2", (128, 2, TP), dt.bfloat16)
    din("ba2_row", (1, TP), dt.bfloat16)
    din("w_dec_g", (128, 2, 4 * HD), dt.bfloat16)
    din("w_dec_gx", (128, 4 * HD), dt.bfloat16)
    din("xi_init", (128, BL), dt.bfloat16)
    din("ident_f32", (128, 128), dt.float32)
    din("w_fh", (128, 2, 1), dt.bfloat16)
    din("consts_b", (128, 4), dt.float32)
    din("ident", (128, 128), dt.bfloat16)
    out_d = nc.declare_dram_parameter("out", [BL], dt.float32, isOutput=True)
    fl_d = nc.declare_dram_parameter("fldump", [128, 4], dt.float32, isOutput=True)

    with TileContext(nc) as tc:
        with (
            tc.tile_pool(name="consts", bufs=1) as cp,
            tc.tile_pool(name="state", bufs=1) as sp,
            tc.tile_pool(name="feat", bufs=8) as fp,
            tc.tile_pool(name="work", bufs=2) as wp,
        ):
            # ---- load weights into SBUF ----
            sb = {}
            for name, shape, dty in [
                ("w_enc_a1", (128, 5, TP), dt.bfloat16),
                ("ba1_row", (1, 384), dt.bfloat16),
                ("w_enc_a2", (128, 3, 128), dt.bfloat16),
                ("b_enc_a2", (128, 1), dt.float32),
                ("w_enc_g", (128, 3, 4 * HE), dt.bfloat16),
                ("b_enc_g_row", (1, 4 * HE), dt.bfloat16),
                ("w_q", (128, 2, 2), dt.bfloat16),
                ("w_dec_a1", (128, 6, HE), dt.bfloat16),
                ("dba1_row", (1, HE), dt.bfloat16),
                ("w_dec_a2", (128, 2, TP), dt.bfloat16),
                ("ba2_row", (1, TP), dt.bfloat16),
                ("w_dec_g", (128, 2, 4 * HD), dt.bfloat16),
                ("w_dec_gx", (128, 4 * HD), dt.bfloat16),
                ("w_fh", (128, 2, 1), dt.bfloat16),
                ("consts_b", (128, 4), dt.float32),
                ("ident", (128, 128), dt.bfloat16),
                ("ident_f32", (128, 128), dt.float32),
                ("tgw", (BL, T), dt.float32),
            ]:
                t = cp.tile(list(shape), dty, tag=name)
                nc.sync.dma_start(out=t, in_=dram[name].ap())
                sb[name] = t

            ones_row = cp.tile([1, 128], dt.bfloat16, tag="ones_row")
            nc.vector.memset(ones_row, 1.0)
            ones_b = cp.tile([1, BL], dt.bfloat16, tag="ones_b")
            nc.vector.memset(ones_b, 1.0)
            zero_bf = cp.tile([128, BL], dt.bfloat16, tag="zero")
            nc.vector.memset(zero_bf, 0.0)

            # persistent big buffers
            hs = cp.tile([128, TP, 2, BL], dt.bfloat16, tag="hs")  # [f, t, half, b]
            qT = cp.tile([128, 3, 2, BL], dt.bfloat16, tag="qT")  # [t'%128, t'//128, {c,f}, b]
            nc.vector.memset(qT, 0.0)
            qcT = cp.tile([128, 384], dt.bfloat16, tag="qcT")  # [b, t'] (padded)
            qfT = cp.tile([128, 384], dt.bfloat16, tag="qfT")

            # encoder state: single bf16 c (contractive recurrence; all-bf16
            # elementwise ops run the DVE in 2x mode)
            c_b = sp.tile([128, 2, BL], dt.bfloat16, tag="c_b")
            nc.vector.memset(c_b, 0.0)

            # ================= encoder =================
            with (
                tc.tile_pool(name="ps_a1", bufs=2, space="PSUM") as ps_a1,
                tc.tile_pool(name="ps_g", bufs=2, space="PSUM") as ps_g,
                tc.tile_pool(name="ps_q", bufs=1, space="PSUM") as ps_q,
                tc.tile_pool(name="ps_fl", bufs=1, space="PSUM") as ps_fl,
            ):
                enxt = {}
                fts = {}

                def enc_prefill(t):
                    # x-part of attn1 + bias rows for step t, off the
                    # critical chain.  stop must ride on a full-128-partition
                    # matmul (psum group state is per-partition).
                    ft = fp.tile([128, BL], dt.bfloat16, tag="ft", name="ft")
                    nc.sync.dma_start(out=ft, in_=dram["featT"].ap()[:, t, :])
                    fts[t] = ft
                    a1 = ps_a1.tile([128, 4, BL], dt.float32, tag="a1", name="a1")
                    for i, (m, mm) in enumerate(((0, 128), (2, 1), (1, 128))):
                        nc.tensor.matmul(
                            a1[:mm, m, :],
                            sb["w_enc_a1"][:, 0, m * 128 : m * 128 + mm],
                            ft,
                            start=(i == 0),
                            stop=False,
                        )
                    for i, (m, mm) in enumerate(((2, 1), (0, 128), (1, 128))):
                        nc.tensor.matmul(
                            a1[:mm, m, :],
                            sb["ba1_row"][0:1, m * 128 : m * 128 + mm],
                            ones_b,
                            start=False,
                            stop=(i == 2),
                        )
                    enxt[t] = a1

                enc_prefill(0)

                for t in range(enc_steps):
                    ft = fts.pop(t)
                    a1 = enxt.pop(t)

                    if t == 0:
                        hp0, hp1 = zero_bf, zero_bf
                    else:
                        hp0, hp1 = hs[:, t - 1, 0, :], hs[:, t - 1, 1, :]
                    rhs_g = [None, hp0, hp1]

                    # gates bias rows first (no deps -> run during prev tail)
                    g8e = ps_g.tile([128, 8, BL], dt.float32, tag="g8e", name="g8e")
                    for j in range(8):
                        nc.tensor.matmul(
                            g8e[:, j, :],
                            sb["b_enc_g_row"][0:1, j * 128 : (j + 1) * 128],
                            ones_b,
                            start=(j in (0, 4)),  # one start per psum bank
                            stop=False,
                        )

                    # attn1: aT [257 -> (128,128,1), b] ; one bank [m0,m1,m2,al]
                    # c-parts first: c_b is ready ~1us before h -> these MMs
                    # fill the prev step's tail PE-idle window
                    a1m = [a1[:, 0, :], a1[:, 1, :], a1[:1, 2, :]]
                    rhs_a1 = {1: hp0, 2: hp1, 3: c_b[:, 0, :], 4: c_b[:, 1, :]}
                    for k in (3, 4, 1, 2):
                        for m, mm in enumerate((128, 128, 1)):
                            nc.tensor.matmul(
                                a1m[m],
                                sb["w_enc_a1"][:, k, m * 128 : m * 128 + mm],
                                rhs_a1[k],
                                start=False,
                                stop=True,
                                skip_group_check=True,
                            )

                    # gates h-part: 4 MMs cover the tanh latency, then attn2
                    # jumps the queue, then the remaining 12
                    def enc_gate_h(j):
                        for k in (1, 2):
                            nc.tensor.matmul(
                                g8e[:, j, :],
                                sb["w_enc_g"][:, k, j * 128 : (j + 1) * 128],
                                rhs_g[k],
                                start=False,
                                stop=(j in (3, 7) and k == 2),
                            )

                    for j in (0, 1):
                        enc_gate_h(j)

                    # tanh (ACT): one call over all 3 m-chunks (m2 rows 1..127
                    # are garbage but never read)
                    aT3 = wp.tile([128, 3, BL], dt.bfloat16, tag="aT3", name="aT3")
                    nc.scalar.activation(out=aT3, in_=a1[:, 0:3, :], func=AF.Tanh)
                    aT = [aT3[:, 0, :], aT3[:, 1, :], aT3[:1, 2, :]]

                    # attn2 + xi = (alpha + ba2) * x_t
                    al_ps = a1[:, 3, :]
                    for k, kk in enumerate((128, 128, 1)):
                        nc.tensor.matmul(
                            al_ps,
                            sb["w_enc_a2"][:kk, k, :],
                            aT[k][:kk, :],
                            start=(k == 0),
                            stop=(k == 2),
                        )

                    for j in (2, 3, 4, 5, 6, 7):
                        enc_gate_h(j)
                    xiT = wp.tile([128, BL], dt.bfloat16, tag="xiT")
                    nc.vector.scalar_tensor_tensor(
                        out=xiT,
                        in0=al_ps,
                        scalar=sb["b_enc_a2"][:, 0:1],
                        in1=ft,
                        op0=OP.add,
                        op1=OP.mult,
                    )

                    # gates xi-part: late accumulate into the closed groups
                    # (has_written bits already set -> HW accumulates; the
                    # group check is sim-only bookkeeping)
                    for j in range(8):
                        nc.tensor.matmul(
                            g8e[:, j, :],
                            sb["w_enc_g"][:, 0, j * 128 : (j + 1) * 128],
                            xiT,
                            start=False,
                            stop=True,
                            skip_group_check=True,
                        )

                    # LSTM elementwise: wide ACTs (bias already in psum);
                    # all-bf16 so the DVE tensor_tensor ops run in 2x mode
                    s_if = wp.tile([128, 4, BL], dt.bfloat16, tag="s_if", name="s_if")
                    s_g = wp.tile([128, 2, BL], dt.bfloat16, tag="s_g", name="s_g")
                    s_o = wp.tile([128, 2, BL], dt.bfloat16, tag="s_o", name="s_o")
                    nc.scalar.activation(out=s_if, in_=g8e[:, 0:4, :], func=AF.Sigmoid)
                    nc.scalar.activation(out=s_g, in_=g8e[:, 4:6, :], func=AF.Tanh)
                    nc.scalar.activation(out=s_o, in_=g8e[:, 6:8, :], func=AF.Sigmoid)
                    p1 = wp.tile([128, 2, BL], dt.bfloat16, tag="p1")
                    t2 = wp.tile([128, 2, BL], dt.bfloat16, tag="t2")
                    nc.vector.tensor_mul(t2, s_if[:, 2:4, :], c_b)
                    nc.vector.tensor_mul(p1, s_if[:, 0:2, :], s_g)
                    nc.vector.tensor_add(c_b, t2, p1)
                    tc_ = wp.tile([128, 2, BL], dt.bfloat16, tag="tc")
                    nc.scalar.activation(out=tc_, in_=c_b, func=AF.Tanh)
                    nc.vector.tensor_mul(hs[:, t, :, :], s_o, tc_)

                    # prefill next step's attn1 x-part, then HAM-warmth
                    # fillers that stream during the LSTM tail's PE idle
                    if t + 1 < enc_steps:
                        enc_prefill(t + 1)
                    flt = ps_fl.tile([128, 512], dt.float32, tag="fl", name="fl")
                    for _ in range(5):
                        nc.tensor.matmul(
                            flt, sb["ident"], sb["w_enc_g"][:, 0, 0:512],
                            start=True, stop=True,
                        )

                    # q rows: q_{c,f}[t] = h_t . W -> [2, b] -> DMA to qT row
                    q_ps = ps_q.tile([2, BL], dt.float32, tag="q_ps")
                    for k in range(2):
                        nc.tensor.matmul(
                            q_ps,
                            sb["w_q"][:, k, :],
                            hs[:, t, k, :],
                            start=(k == 0),
                            stop=(k == 1),
                        )
                    q_row = fp.tile([2, BL], dt.bfloat16, tag="q_row")
                    nc.vector.tensor_copy(q_row, q_ps)
                    nc.sync.dma_start(
                        out=qT[t % 128 : t % 128 + 1, t // 128, :, :], in_=q_row
                    )

            # ======= transpose q to batch-major qcT/qfT [b, t'] =======
            with tc.tile_pool(name="ps_tr", bufs=2, space="PSUM") as ps_tr:
                for chunk in range(3):
                    for cf in range(2):
                        tp_ps = ps_tr.tile([128, 128], dt.bfloat16, tag="tp")
                        nc.tensor.transpose(
                            tp_ps, qT[:, chunk, cf, :], sb["ident"]
                        )
                        dst = qcT if cf == 0 else qfT
                        nc.vector.tensor_copy(
                            dst[:, chunk * 128 : (chunk + 1) * 128], tp_ps
                        )

            # ================= decoder =================
            hdT = sp.tile([128, 2, BL], dt.bfloat16, tag="hdT")  # h' = 2h
            # single bf16 c-state (c' = 2c): the c recurrence is contractive
            # so bf16 rounding stays bounded, and all-bf16 elementwise ops
            # run the DVE in 2x mode
            cd_b = sp.tile([128, 2, BL], dt.bfloat16, tag="cd_b")
            nc.vector.memset(hdT, 0.0)
            nc.vector.memset(cd_b, 0.0)
            # [xi_nc; 1; 0...] padded to 128 partitions (matches padded w_dec_gx)
            xiT2 = sp.tile([128, BL], dt.bfloat16, tag="xiT2")
            nc.sync.dma_start(out=xiT2, in_=dram["xi_init"].ap())
            o_col = sp.tile([128, 1], dt.float32, tag="o_col")

            with (
                tc.tile_pool(name="ps_da1", bufs=2, space="PSUM") as ps_da1,
                tc.tile_pool(name="ps_s", bufs=2, space="PSUM") as ps_s,
                tc.tile_pool(name="ps_dg", bufs=1, space="PSUM") as ps_dg,
                tc.tile_pool(name="ps_dx", bufs=1, space="PSUM") as ps_dx,
            ):
                ps_fl2 = ps_dx  # fillers share the ps_dx bank
                nxt = {}

                def prefill(t):
                    da1 = ps_da1.tile([128, 2, BL], dt.float32, tag="da1", name="da1")
                    for m in range(2):
                        for k in range(2):  # feat chunks (4, 5)
                            nc.tensor.matmul(
                                da1[:, m, :],
                                sb["w_dec_a1"][:, 4 + k, m * 128 : (m + 1) * 128],
                                hs[:, t, k, :],
                                start=(m == 0 and k == 0),
                                stop=False,
                            )
                    for m in range(2):  # attn1 bias rows
                        nc.tensor.matmul(
                            da1[:, m, :],
                            sb["dba1_row"][0:1, m * 128 : (m + 1) * 128],
                            ones_b,
                            start=False,
                            stop=(m == 1),
                        )
                    nxt[t] = da1

                prefill(0)

                for t in range(dec_steps):
                    da1 = nxt.pop(t)
                    # attn2 bias row first (no deps; ps_s bufs=2 so no WAR
                    # stall at the PE queue head)
                    s_ps = ps_s.tile(
                        [128, TP], dt.float32, tag="s_ps", padded_shape=[128, 512]
                    )
                    nc.tensor.matmul(
                        s_ps, ones_row, sb["ba2_row"], start=True, stop=False
                    )

                    # attn1 h/c accumulate: c-parts FIRST (cd_b is ready ~1us
                    # before hdT -> these 4 MMs break up the tail PE-idle gap)
                    rhs_a1 = [
                        cd_b[:, 0, :],
                        cd_b[:, 1, :],
                        hdT[:, 0, :],
                        hdT[:, 1, :],
                    ]
                    wk_a1 = [2, 3, 0, 1]  # w_dec_a1 k-chunk for each rhs
                    for k in range(4):
                        for m in range(2):
                            nc.tensor.matmul(
                                da1[:, m, :],
                                sb["w_dec_a1"][:, wk_a1[k], m * 128 : (m + 1) * 128],
                                rhs_a1[k],
                                start=False,
                                stop=True,
                                skip_group_check=True,
                            )

                    # gates h-part: first 4 MMs cover the aTd-tanh latency,
                    # then attn2 jumps the queue, then the remaining 12
                    g8d = ps_dg.tile([128, 8, BL], dt.float32, tag="g8d", name="g8d")

                    def gate_h(j):
                        for k in range(2):
                            nc.tensor.matmul(
                                g8d[:, j, :],
                                sb["w_dec_g"][:, k, j * 128 : (j + 1) * 128],
                                hdT[:, k, :],
                                start=(j in (0, 4) and k == 0),
                                stop=(j in (3, 7) and k == 1),
                            )

                    for j in (0, 1):
                        gate_h(j)

                    # attn1 tanh (bias already in psum via prefill)
                    aTd = wp.tile([128, 2, BL], dt.bfloat16, tag="aTd")
                    nc.scalar.activation(out=aTd, in_=da1, func=AF.Tanh)

                    # attn2, batch-major: s[b, t']
                    for k in range(2):
                        nc.tensor.matmul(
                            s_ps,
                            aTd[:, k, :],
                            sb["w_dec_a2"][:, k, :],
                            start=False,
                            stop=(k == 1),
                        )

                    for j in (2, 3, 4, 5, 6, 7):
                        gate_h(j)

                    # prefill next step's attn1 feat part while the softmax
                    # chain runs on ACT/DVE, plus warmth fillers to keep the
                    # PE busy-density above the HAM un-throttle threshold
                    if t + 1 < dec_steps:
                        prefill(t + 1)
                    flt2 = ps_fl2.tile([128, 512], dt.float32, tag="fl2", name="fl2")
                    for fk in range(2):
                        nc.tensor.matmul(
                            flt2[:, 0:256],
                            sb["ident"],
                            sb["w_dec_g"][:, 0, (fk % 4) * 256 : (fk % 4) * 256 + 256],
                            start=True,
                            stop=True,
                        )

                    # softmax pieces: e, Z, dot_c, xi
                    e_sb = wp.tile([128, TP], dt.bfloat16, tag="e_sb")
                    z_t = wp.tile([128, 1], dt.float32, tag="z_t")
                    nc.scalar.activation(
                        out=e_sb, in_=s_ps, func=AF.Exp, accum_out=z_t
                    )
                    eq = wp.tile([128, TP], dt.bfloat16, tag="eq")
                    rz = wp.tile([128, 1], dt.float32, tag="rz")
                    dot_c = wp.tile([128, 1], dt.float32, tag="dot_c")
                    nc.vector.tensor_mul(eq, e_sb, qcT[:, 0:TP])
                    nc.vector.reciprocal(rz, z_t)
                    nc.vector.tensor_reduce(
                        out=dot_c, in_=eq, axis=mybir.AxisListType.X, op=OP.add
                    )
                    # xi = dot_c/Z + y*Wc00 (tgw precomputed on host)
                    xi_col = wp.tile([128, 1], dt.bfloat16, tag="xi_col")
                    nc.vector.scalar_tensor_tensor(
                        out=xi_col,
                        in0=dot_c,
                        scalar=rz,
                        in1=sb["tgw"][:, t : t + 1],
                        op0=OP.mult,
                        op1=OP.add,
                    )
                    # xi back to row layout: regular matmul against identity
                    # (out[0, n] = sum_k xi[k] * I[k, n] = xi[n])
                    xi_ps = ps_dx.tile([1, 128], dt.float32, tag="xi_ps")
                    nc.tensor.matmul(
                        xi_ps, xi_col, sb["ident"], start=True, stop=True
                    )
                    nc.vector.tensor_copy(xiT2[0:1, :], xi_ps[0:1, :])

                    # gates [xi; 1] chunk (bias folded): late accumulate
                    for j in range(8):
                        nc.tensor.matmul(
                            g8d[:, j, :],
                            sb["w_dec_gx"][:, j * 128 : (j + 1) * 128],
                            xiT2,
                            start=False,
                            stop=True,
                            skip_group_check=True,
                        )

                    # LSTM elementwise (tanh-form): tanh in two halves [i,f]
                    # then [g,o] so the DVE t2 op overlaps the second ACT.
                    # Everything bf16 so the DVE stt ops run in 2x mode.
                    th8 = wp.tile([128, 8, BL], dt.bfloat16, tag="th8")
                    nc.scalar.activation(
                        out=th8[:, 0:4, :], in_=g8d[:, 0:4, :], func=AF.Tanh, scale=0.5
                    )
                    nc.scalar.activation(
                        out=th8[:, 4:8, :], in_=g8d[:, 4:8, :], func=AF.Tanh, scale=0.5
                    )
                    # tail fillers anchored on th8 halves (fire mid-tail)
                    nc.tensor.matmul(
                        flt2[:, 0:128], sb["ident"], th8[:, 0, :],
                        start=True, stop=True,
                    )
                    nc.tensor.matmul(
                        flt2[:, 0:128], sb["ident"], th8[:, 4, :],
                        start=True, stop=True,
                    )
                    p1 = wp.tile([128, 2, BL], dt.bfloat16, tag="dp1")
                    t2 = wp.tile([128, 2, BL], dt.bfloat16, tag="dt2")
                    nc.vector.scalar_tensor_tensor(
                        out=t2,
                        in0=th8[:, 2:4, :],
                        scalar=1.0,
                        in1=cd_b,
                        op0=OP.add,
                        op1=OP.mult,
                    )
                    nc.vector.scalar_tensor_tensor(
                        out=p1,
                        in0=th8[:, 0:2, :],
                        scalar=1.0,
                        in1=th8[:, 4:6, :],
                        op0=OP.add,
                        op1=OP.mult,
                    )
                    # c'_new = 0.5 * t2 + p1 (overwrites the bf16 c-state)
                    nc.vector.scalar_tensor_tensor(
                        out=cd_b,
                        in0=t2,
                        scalar=0.5,
                        in1=p1,
                        op0=OP.mult,
                        op1=OP.add,
                    )
                    # fillers anchored on cd_b + one free 512 behind them
                    nc.tensor.matmul(
                        flt2[:, 0:128], sb["ident"], cd_b[:, 0, :],
                        start=True, stop=True,
                    )
                    nc.tensor.matmul(
                        flt2[:, 0:128], sb["ident"], cd_b[:, 1, :],
                        start=True, stop=True,
                    )
                    nc.tensor.matmul(
                        flt2, sb["ident"], sb["w_dec_g"][:, 0, 0:512],
                        start=True, stop=True,
                    )
                    th_c = wp.tile([128, 2, BL], dt.bfloat16, tag="th_c")
                    nc.scalar.activation(
                        out=th_c, in_=cd_b, func=AF.Tanh, scale=0.5
                    )
                    # h' = (th_o + 1) * th_c
                    nc.vector.scalar_tensor_tensor(
                        out=hdT,
                        in0=th8[:, 6:8, :],
                        scalar=1.0,
                        in1=th_c,
                        op0=OP.add,
                        op1=OP.mult,
                    )

                    if t == dec_steps - 1:
                        # dot_f ; out = hd.Wf + dot_f/Z + bf  (batch-major)
                        eq2 = wp.tile([128, TP], dt.bfloat16, tag="eq2")
                        dot_f = wp.tile([128, 1], dt.float32, tag="dot_f")
                        nc.vector.tensor_mul(eq2, e_sb, qfT[:, 0:TP])
                        nc.vector.tensor_reduce(
                            out=dot_f, in_=eq2, axis=mybir.AxisListType.X, op=OP.add
                        )
                        df = wp.tile([128, 1], dt.float32, tag="df")
                        nc.vector.tensor_mul(df, dot_f, rz)
                        fin_ps = flt2[:, 0:1]  # reuse the filler psum bank
                        for k in range(2):
                            nc.tensor.matmul(
                                fin_ps,
                                hdT[:, k, :],
                                sb["w_fh"][:, k, :],
                                start=(k == 0),
                                stop=(k == 1),
                            )
                        nc.vector.scalar_tensor_tensor(
                            out=o_col,
                            in0=df,
                            scalar=sb["consts_b"][:, 1:2],
                            in1=fin_ps,
                            op0=OP.add,
                            op1=OP.add,
                        )
                        nc.sync.dma_start(out=out_d.ap(), in_=o_col)

                # consume the filler psum so nothing upstream is elided
                fl_sb = sp.tile([128, 4], dt.float32, tag="fl_sb")
                nc.vector.tensor_copy(fl_sb, flt2[:, 0:4])
                nc.sync.dma_start(out=fl_d.ap(), in_=fl_sb)

    nc.finalize()
    return nc


def _get_nc():
    if "nc" not in _CACHE:
        _CACHE["nc"] = _build()
    return _CACHE["nc"]


def _run(inputs, **kw):
    from concourse.bass_utils import run_bass_kernel_spmd

    shared, per_core = _pack_inputs(inputs)
    nc = _get_nc()
    in_maps = []
    for c in range(NCORES):
        m = dict(shared)
        m.update(per_core[c])
        in_maps.append(m)
    res = run_bass_kernel_spmd(nc, in_maps, list(range(NCORES)), **kw)
    out = np.concatenate([np.asarray(res.results[c]["out"]) for c in range(NCORES)])
    return out.astype(np.float32).reshape(B, 1), res


def kernel(**inputs):
    return _run(inputs)[0]



# revision 64
# speedup vs baseline: 1.1811x; 1.1811x over previous
"""DARNN (dual-attention RNN) Trainium2 Bass kernel — v5.

Strategy (pure data parallel, 8 cores, B=1024 -> 128 samples/core):

Activations are feature-major on-chip: x[b, f] lives in SBUF as xT[f, b]
(features on partitions, local batch on the free dim). Matmuls contract
over partitions with pre-transposed bf16 weights stationary.

v2 foundation:
- Decoder softmax is batch-major: attn2 uses the (feature-major) tanh
  output as the *stationary* operand, producing scores [b, 257] in one
  PSUM bank (bias folded in via a ones-row matmul). A single Exp
  activation with accum_out yields e and Z; xi returns to row layout
  via one PE transpose.  (NOTE: the fused DVE tensor_tensor_reduce
  hangs the device — NRT_EXEC_UNIT_UNRECOVERABLE — use separate
  tensor_mul + tensor_reduce.)
- All decoder sigmoids are rewritten as 0.5*tanh(x/2)+0.5 with the 0.5/2
  factors folded into packed weights (h'=2h, c'=2c representation), so
  the decoder only ever uses {Tanh, Exp} -> zero ACT table reloads.
- Decoder gate biases (+ comb_fc bias) are folded into the [xi; 1]
  matmul chunk; attn1 feat contribution for t+1 prefilled during t.

v3-v5 changes (trace-driven):
- ps_s PSUM pool bufs=2: the attn2-bias matmul no longer WAR-stalls at
  the head of the PE FIFO (~2.3us/step).
- Decoder PE issue order: attn2 placed right after the aTd tanh with
  only 4 gates-h matmuls covering the tanh latency (was: all 16 ahead
  of it); remaining gates-h + prefill fill the softmax window.
- attn1 h/c accumulation issues the c-dependent matmuls first: cd_b is
  ready ~1us before hdT, so they break up the LSTM-tail PE-idle gap.
- gates-[xi;1] stationary zero-padded from K=2 to K=128 -> FWL applies.
- Decoder gate tanh split into [i,f] / [g,o] halves so the DVE t2 op
  overlaps the second half.
- HAM warmth economics (measured): the PE un-throttles (1.2->2.4 GHz)
  only when the activity window stays densely busy, so junk "filler"
  matmuls PAY: removing them makes the whole kernel ~15% slower.
  Decoder fillers are data-anchored on th8/cd_b so the scheduler cannot
  hoist them out of the tail, and a final PSUM->DRAM dump ("fldump")
  keeps them live.  Encoder keeps the v2 5x512 tail fillers.
- Run-to-run HW variance is ~0.5-0.9ms (device throttle-state drift);
  measure twice before believing any scheduling delta.
"""

import numpy as np
import ml_dtypes

B, T, NF, HE, HD = 1024, 256, 128, 256, 256
NCORES = 8
BL = B // NCORES  # 128 local batch
TP = T + 1  # 257
BF16 = ml_dtypes.bfloat16

_CACHE = {}


def _bf(x):
    return np.ascontiguousarray(np.asarray(x).astype(BF16))


def _f32(x):
    return np.ascontiguousarray(np.asarray(x).astype(np.float32))


def _pack_inputs(inputs):
    """Pack weights/biases into per-SBUF-tile layouts (shared across cores)."""
    f = {k: np.asarray(v, dtype=np.float32) for k, v in inputs.items()}
    p = {}

    # --- encoder attn1: a = tanh(cat[x,h,c] @ Wa1.T + ba1) ---
    # lhsT tile [128, 5, 257]: [p, k, m] = Wa1[m, k*128+p]
    wa1 = f["enc_Wa1"]  # [257, 640]
    p["w_enc_a1"] = _bf(wa1.T.reshape(5, 128, TP).transpose(1, 0, 2))
    b1 = np.zeros((1, 384), np.float32)
    b1[0, :TP] = f["enc_ba1"]  # bias row for the psum bias-matmul
    p["ba1_row"] = _bf(b1)

    # --- encoder attn2: alpha = a @ Wa2.T + ba2 ---  K=257 (3 chunks), M=128
    wa2 = f["enc_Wa2"]  # [128, 257]
    w = np.zeros((128, 3, 128), np.float32)
    w2 = wa2.T  # [257, 128]
    w[:, 0, :] = w2[0:128]
    w[:, 1, :] = w2[128:256]
    w[0, 2, :] = w2[256]
    p["w_enc_a2"] = _bf(w)
    p["b_enc_a2"] = _f32(f["enc_ba2"].reshape(128, 1))

    # --- encoder gates: g = Wih@xi + Whh@h + b ---  K chunks: [xi, h0, h1]
    # column order j = [i0,i1,f0,f1,g0,g1,o0,o1] (natural torch order)
    w = np.zeros((128, 3, 4 * HE), np.float32)
    w[:, 0, :] = f["enc_Wih"].T  # [128, 1024]
    w[:, 1, :] = f["enc_Whh"].T[0:128]
    w[:, 2, :] = f["enc_Whh"].T[128:256]
    p["w_enc_g"] = _bf(w)
    bsum = f["enc_bih"] + f["enc_bhh"]
    p["b_enc_g_row"] = _bf(bsum.reshape(1, 4 * HE))

    # --- q projections: q_c = h . Wc[0,1:], q_f = h . Wf[0,HD:] ---
    w = np.zeros((128, 2, 2), np.float32)
    w[:, 0, 0] = f["dec_Wc"][0, 1 : 1 + 128]
    w[:, 1, 0] = f["dec_Wc"][0, 129 : 1 + 256]
    w[:, 0, 1] = f["dec_Wf"][0, HD : HD + 128]
    w[:, 1, 1] = f["dec_Wf"][0, HD + 128 : HD + 256]
    p["w_q"] = _bf(w)

    # --- decoder attn1: a = tanh(cat[h,c,feat] @ Wa1.T + ba1) --- K=768 (6)
    # decoder h,c are stored as h'=2h, c'=2c -> scale those k-chunks by 0.5
    wa1d = f["dec_Wa1"].copy()  # [256, 768]
    wa1d[:, 0:512] *= 0.5  # h and c columns
    p["w_dec_a1"] = _bf(wa1d.T.reshape(6, 128, HE).transpose(1, 0, 2))
    p["dba1_row"] = _bf(f["dec_ba1"].reshape(1, HE))

    # --- decoder attn2 (batch-major): s[b, t'] = a.T @ Wa2dT + ba2 ---
    # moving operand [p=feat chunk, k, n=t'] = Wa2d[n, k*128+p]
    wa2d = f["dec_Wa2"]  # [257, 256]
    p["w_dec_a2"] = _bf(wa2d.T.reshape(2, 128, TP).transpose(1, 0, 2))
    ba2r = np.zeros((1, TP), np.float32)
    ba2r[0, :] = f["dec_ba2"]
    p["ba2_row"] = _bf(ba2r)

    # --- decoder gates ---
    # tanh-form LSTM: i,f,o gates become tanh(0.5*(pre+b)); g stays tanh.
    # Fold: h' = 2h -> Whh columns *0.5 ; g-gate pre-act scaled *2 so a
    # uniform ACT scale=0.5 works for the whole tile.
    sgate = np.ones((4 * HD,), np.float32)
    sgate[512:768] = 2.0  # g-gate columns
    whh = f["dec_Whh"].T * 0.5  # [256, 1024] (h' fold)
    w = np.zeros((128, 2, 4 * HD), np.float32)
    w[:, 0, :] = whh[0:128] * sgate
    w[:, 1, :] = whh[128:256] * sgate
    p["w_dec_g"] = _bf(w)
    # k=2 chunk [xi_nc; 1] with xi_nc = y*Wc00 + dot_c/Z  (bc folded here)
    wih = f["dec_Wih"][:, 0]  # [1024]
    bsum = f["dec_bih"] + f["dec_bhh"] + wih * f["dec_bc"][0]
    gx = np.zeros((128, 4 * HD), np.float32)  # K padded to 128 -> FWL
    gx[0, :] = wih * sgate
    gx[1, :] = bsum * sgate
    p["w_dec_gx"] = _bf(gx)

    # --- final: out = hd . Wf[0,:HD] + dot_f/Z + bf ---  (hd' = 2hd fold)
    w = np.zeros((128, 2, 1), np.float32)
    w[:, 0, 0] = f["dec_Wf"][0, 0:128] * 0.5
    w[:, 1, 0] = f["dec_Wf"][0, 128:256] * 0.5
    p["w_fh"] = _bf(w)

    # --- broadcast consts: [bc, bf, Wc00, 0] replicated over partitions ---
    cb = np.zeros((128, 4), np.float32)
    cb[:, 0] = f["dec_bc"][0]
    cb[:, 1] = f["dec_bf"][0]
    cb[:, 2] = f["dec_Wc"][0, 0]
    p["consts_b"] = _f32(cb)

    # --- identity for PE transposes ---
    p["ident"] = _bf(np.eye(128, dtype=np.float32))

    # --- xiT2 init: row 1 = ones, rest 0 (row 0 rewritten per step) ---
    xi0 = np.zeros((128, BL), np.float32)
    xi0[1, :] = 1.0
    p["xi_init"] = _bf(xi0)

    # --- f32 identity (stationary for f32-moving filler matmuls) ---
    p["ident_f32"] = _f32(np.eye(128, dtype=np.float32))

    # --- per-core tensors ---
    feat = f["feat"]  # [B, 257, 128]
    # tgw = target * Wc00, host-precomputed (b-major)
    tgw = f["target"] * f["dec_Wc"][0, 0]
    per_core = []
    for c in range(NCORES):
        sl = slice(c * BL, (c + 1) * BL)
        featT = _bf(feat[sl].transpose(2, 1, 0))  # [f=128, t=257, b=128]
        per_core.append({"featT": featT, "tgw": _f32(tgw[sl])})
    return p, per_core


def _build(enc_steps=TP, dec_steps=T):
    import concourse.mybir as mybir
    from concourse import bacc
    from concourse.tile import TileContext

    dt = mybir.dt
    AF = mybir.ActivationFunctionType
    OP = mybir.AluOpType

    nc = bacc.Bacc("TRN2")

    # ---- DRAM parameters ----
    dram = {}

    def din(name, shape, dtype):
        dram[name] = nc.declare_dram_parameter(name, list(shape), dtype, isOutput=False)

    din("featT", (128, TP, BL), dt.bfloat16)
    din("tgw", (BL, T), dt.float32)
    din("w_enc_a1", (128, 5, TP), dt.bfloat16)
    din("ba1_row", (1, 384), dt.bfloat16)
    din("w_enc_a2", (128, 3, 128), dt.bfloat16)
    din("b_enc_a2", (128, 1), dt.float32)
    din("w_enc_g", (128, 3, 4 * HE), dt.bfloat16)
    din("b_enc_g_row", (1, 4 * HE), dt.bfloat16)
    din("w_q", (128, 2, 2), dt.bfloat16)
    din("w_dec_a1", (128, 6, HE), dt.bfloat16)
    din("dba1_row", (1, HE), dt.bfloat16)
    din("w_dec_a2", (128, 2, TP), dt.bfloat16)
    din("ba2_row", (1, TP), dt.bfloat16)
    din("w_dec_g", (128, 2, 4 * HD), dt.bfloat16)
    din("w_dec_gx", (128, 4 * HD), dt.bfloat16)
    din("xi_init", (128, BL), dt.bfloat16)
    din("ident_f32", (128, 128), dt.float32)
    din("w_fh", (128, 2, 1), dt.bfloat16)
    din("consts_b", (128, 4), dt.float32)
    din("ident", (128, 128), dt.bfloat16)
    out_d = nc.declare_dram_parameter("out", [BL], dt.float32, isOutput=True)
    fl_d = nc.declare_dram_parameter("fldump", [128, 4], dt.float32, isOutput=True)

    with TileContext(nc) as tc:
        with (
            tc.tile_pool(name="consts", bufs=1) as cp,
            tc.tile_pool(name="state", bufs=1) as sp,
            tc.tile_pool(name="feat", bufs=8) as fp,
            tc.tile_pool(name="work", bufs=2) as wp,
        ):
            # ---- load weights into SBUF ----
            sb = {}
            for name, shape, dty in [
                ("w_enc_a1", (128, 5, TP), dt.bfloat16),
                ("ba1_row", (1, 384), dt.bfloat16),
                ("w_enc_a2", (128, 3, 128), dt.bfloat16),
                ("b_enc_a2", (128, 1), dt.float32),
                ("w_enc_g", (128, 3, 4 * HE), dt.bfloat16),
                ("b_enc_g_row", (1, 4 * HE), dt.bfloat16),
                ("w_q", (128, 2, 2), dt.bfloat16),
                ("w_dec_a1", (128, 6, HE), dt.bfloat16),
                ("dba1_row", (1, HE), dt.bfloat16),
                ("w_dec_a2", (128, 2, TP), dt.bfloat16),
                ("ba2_row", (1, TP), dt.bfloat16),
                ("w_dec_g", (128, 2, 4 * HD), dt.bfloat16),
                ("w_dec_gx", (128, 4 * HD), dt.bfloat16),
                ("w_fh", (128, 2, 1), dt.bfloat16),
                ("consts_b", (128, 4), dt.float32),
                ("ident", (128, 128), dt.bfloat16),
                ("ident_f32", (128, 128), dt.float32),
                ("tgw", (BL, T), dt.float32),
            ]:
                t = cp.tile(list(shape), dty, tag=name)
                nc.sync.dma_start(out=t, in_=dram[name].ap())
                sb[name] = t

            ones_row = cp.tile([1, 128], dt.bfloat16, tag="ones_row")
            nc.vector.memset(ones_row, 1.0)
            ones_b = cp.tile([1, BL], dt.bfloat16, tag="ones_b")
            nc.vector.memset(ones_b, 1.0)
            zero_bf = cp.tile([128, BL], dt.bfloat16, tag="zero")
            nc.vector.memset(zero_bf, 0.0)

            # persistent big buffers
            hs = cp.tile([128, TP, 2, BL], dt.bfloat16, tag="hs")  # [f, t, half, b]
            qT = cp.tile([128, 3, 2, BL], dt.bfloat16, tag="qT")  # [t'%128, t'//128, {c,f}, b]
            nc.vector.memset(qT, 0.0)
            qcT = cp.tile([128, 384], dt.bfloat16, tag="qcT")  # [b, t'] (padded)
            qfT = cp.tile([128, 384], dt.bfloat16, tag="qfT")

            # encoder state
            c_f = sp.tile([128, 2, BL], dt.float32, tag="c_f")
            c_b = sp.tile([128, 2, BL], dt.bfloat16, tag="c_b")
            nc.vector.memset(c_f, 0.0)
            nc.vector.memset(c_b, 0.0)

            # ================= encoder =================
            with (
                tc.tile_pool(name="ps_a1", bufs=2, space="PSUM") as ps_a1,
                tc.tile_pool(name="ps_g", bufs=2, space="PSUM") as ps_g,
                tc.tile_pool(name="ps_q", bufs=1, space="PSUM") as ps_q,
                tc.tile_pool(name="ps_fl", bufs=1, space="PSUM") as ps_fl,
            ):
                enxt = {}
                fts = {}

                def enc_prefill(t):
                    # x-part of attn1 + bias rows for step t, off the
                    # critical chain.  stop must ride on a full-128-partition
                    # matmul (psum group state is per-partition).
                    ft = fp.tile([128, BL], dt.bfloat16, tag="ft", name="ft")
                    nc.sync.dma_start(out=ft, in_=dram["featT"].ap()[:, t, :])
                    fts[t] = ft
                    a1 = ps_a1.tile([128, 4, BL], dt.float32, tag="a1", name="a1")
                    for i, (m, mm) in enumerate(((0, 128), (2, 1), (1, 128))):
                        nc.tensor.matmul(
                            a1[:mm, m, :],
                            sb["w_enc_a1"][:, 0, m * 128 : m * 128 + mm],
                            ft,
                            start=(i == 0),
                            stop=False,
                        )
                    for i, (m, mm) in enumerate(((2, 1), (0, 128), (1, 128))):
                        nc.tensor.matmul(
                            a1[:mm, m, :],
                            sb["ba1_row"][0:1, m * 128 : m * 128 + mm],
                            ones_b,
                            start=False,
                            stop=(i == 2),
                        )
                    enxt[t] = a1

                enc_prefill(0)

                for t in range(enc_steps):
                    ft = fts.pop(t)
                    a1 = enxt.pop(t)

                    if t == 0:
                        hp0, hp1 = zero_bf, zero_bf
                    else:
                        hp0, hp1 = hs[:, t - 1, 0, :], hs[:, t - 1, 1, :]
                    rhs_g = [None, hp0, hp1]

                    # gates bias rows first (no deps -> run during prev tail)
                    g8e = ps_g.tile([128, 8, BL], dt.float32, tag="g8e", name="g8e")
                    for j in range(8):
                        nc.tensor.matmul(
                            g8e[:, j, :],
                            sb["b_enc_g_row"][0:1, j * 128 : (j + 1) * 128],
                            ones_b,
                            start=(j in (0, 4)),  # one start per psum bank
                            stop=False,
                        )

                    # attn1: aT [257 -> (128,128,1), b] ; one bank [m0,m1,m2,al]
                    # c-parts first: c_b is ready ~1us before h -> these MMs
                    # fill the prev step's tail PE-idle window
                    a1m = [a1[:, 0, :], a1[:, 1, :], a1[:1, 2, :]]
                    rhs_a1 = {1: hp0, 2: hp1, 3: c_b[:, 0, :], 4: c_b[:, 1, :]}
                    for k in (3, 4, 1, 2):
                        for m, mm in enumerate((128, 128, 1)):
                            nc.tensor.matmul(
                                a1m[m],
                                sb["w_enc_a1"][:, k, m * 128 : m * 128 + mm],
                                rhs_a1[k],
                                start=False,
                                stop=True,
                                skip_group_check=True,
                            )

                    # gates h-part: 4 MMs cover the tanh latency, then attn2
                    # jumps the queue, then the remaining 12
                    def enc_gate_h(j):
                        for k in (1, 2):
                            nc.tensor.matmul(
                                g8e[:, j, :],
                                sb["w_enc_g"][:, k, j * 128 : (j + 1) * 128],
                                rhs_g[k],
                                start=False,
                                stop=(j in (3, 7) and k == 2),
                            )

                    for j in (0, 1):
                        enc_gate_h(j)

                    # tanh (ACT): one call over all 3 m-chunks (m2 rows 1..127
                    # are garbage but never read)
                    aT3 = wp.tile([128, 3, BL], dt.bfloat16, tag="aT3", name="aT3")
                    nc.scalar.activation(out=aT3, in_=a1[:, 0:3, :], func=AF.Tanh)
                    aT = [aT3[:, 0, :], aT3[:, 1, :], aT3[:1, 2, :]]

                    # attn2 + xi = (alpha + ba2) * x_t
                    al_ps = a1[:, 3, :]
                    for k, kk in enumerate((128, 128, 1)):
                        nc.tensor.matmul(
                            al_ps,
                            sb["w_enc_a2"][:kk, k, :],
                            aT[k][:kk, :],
                            start=(k == 0),
                            stop=(k == 2),
                        )

                    for j in (2, 3, 4, 5, 6, 7):
                        enc_gate_h(j)
                    xiT = wp.tile([128, BL], dt.bfloat16, tag="xiT")
                    nc.vector.scalar_tensor_tensor(
                        out=xiT,
                        in0=al_ps,
                        scalar=sb["b_enc_a2"][:, 0:1],
                        in1=ft,
                        op0=OP.add,
                        op1=OP.mult,
                    )

                    # gates xi-part: late accumulate into the closed groups
                    # (has_written bits already set -> HW accumulates; the
                    # group check is sim-only bookkeeping)
                    for j in range(8):
                        nc.tensor.matmul(
                            g8e[:, j, :],
                            sb["w_enc_g"][:, 0, j * 128 : (j + 1) * 128],
                            xiT,
                            start=False,
                            stop=True,
                            skip_group_check=True,
                        )

                    # LSTM elementwise: wide ACTs (bias already in psum)
                    s_if = wp.tile([128, 4, BL], dt.float32, tag="s_if", name="s_if")
                    s_g = wp.tile([128, 2, BL], dt.float32, tag="s_g", name="s_g")
                    s_o = wp.tile([128, 2, BL], dt.float32, tag="s_o", name="s_o")
                    nc.scalar.activation(out=s_if, in_=g8e[:, 0:4, :], func=AF.Sigmoid)
                    nc.scalar.activation(out=s_g, in_=g8e[:, 4:6, :], func=AF.Tanh)
                    nc.scalar.activation(out=s_o, in_=g8e[:, 6:8, :], func=AF.Sigmoid)
                    p1 = wp.tile([128, 2, BL], dt.float32, tag="p1")
                    t2 = wp.tile([128, 2, BL], dt.float32, tag="t2")
                    nc.vector.tensor_mul(t2, s_if[:, 2:4, :], c_f)
                    nc.vector.tensor_mul(p1, s_if[:, 0:2, :], s_g)
                    nc.vector.tensor_add(c_f, t2, p1)
                    nc.vector.tensor_copy(c_b, c_f)
                    tc_ = wp.tile([128, 2, BL], dt.float32, tag="tc")
                    nc.scalar.activation(out=tc_, in_=c_f, func=AF.Tanh)
                    nc.vector.tensor_mul(hs[:, t, :, :], s_o, tc_)

                    # prefill next step's attn1 x-part, then HAM-warmth
                    # fillers that stream during the LSTM tail's PE idle
                    if t + 1 < enc_steps:
                        enc_prefill(t + 1)
                    flt = ps_fl.tile([128, 512], dt.float32, tag="fl", name="fl")
                    for _ in range(5):
                        nc.tensor.matmul(
                            flt, sb["ident"], sb["w_enc_g"][:, 0, 0:512],
                            start=True, stop=True,
                        )

                    # q rows: q_{c,f}[t] = h_t . W -> [2, b] -> DMA to qT row
                    q_ps = ps_q.tile([2, BL], dt.float32, tag="q_ps")
                    for k in range(2):
                        nc.tensor.matmul(
                            q_ps,
                            sb["w_q"][:, k, :],
                            hs[:, t, k, :],
                            start=(k == 0),
                            stop=(k == 1),
                        )
                    q_row = fp.tile([2, BL], dt.bfloat16, tag="q_row")
                    nc.vector.tensor_copy(q_row, q_ps)
                    nc.sync.dma_start(
                        out=qT[t % 128 : t % 128 + 1, t // 128, :, :], in_=q_row
                    )

            # ======= transpose q to batch-major qcT/qfT [b, t'] =======
            with tc.tile_pool(name="ps_tr", bufs=2, space="PSUM") as ps_tr:
                for chunk in range(3):
                    for cf in range(2):
                        tp_ps = ps_tr.tile([128, 128], dt.bfloat16, tag="tp")
                        nc.tensor.transpose(
                            tp_ps, qT[:, chunk, cf, :], sb["ident"]
                        )
                        dst = qcT if cf == 0 else qfT
                        nc.vector.tensor_copy(
                            dst[:, chunk * 128 : (chunk + 1) * 128], tp_ps
                        )

            # ================= decoder =================
            hdT = sp.tile([128, 2, BL], dt.bfloat16, tag="hdT")  # h' = 2h
            cd_f = sp.tile([128, 2, BL], dt.float32, tag="cd_f")  # c' = 2c
            cd_b = sp.tile([128, 2, BL], dt.bfloat16, tag="cd_b")
            nc.vector.memset(hdT, 0.0)
            nc.vector.memset(cd_f, 0.0)
            nc.vector.memset(cd_b, 0.0)
            # [xi_nc; 1; 0...] padded to 128 partitions (matches padded w_dec_gx)
            xiT2 = sp.tile([128, BL], dt.bfloat16, tag="xiT2")
            nc.sync.dma_start(out=xiT2, in_=dram["xi_init"].ap())
            o_col = sp.tile([128, 1], dt.float32, tag="o_col")

            with (
                tc.tile_pool(name="ps_da1", bufs=2, space="PSUM") as ps_da1,
                tc.tile_pool(name="ps_s", bufs=2, space="PSUM") as ps_s,
                tc.tile_pool(name="ps_dg", bufs=1, space="PSUM") as ps_dg,
                tc.tile_pool(name="ps_dx", bufs=1, space="PSUM") as ps_dx,
            ):
                ps_fl2 = ps_dx  # fillers share the ps_dx bank
                nxt = {}

                def prefill(t):
                    da1 = ps_da1.tile([128, 2, BL], dt.float32, tag="da1", name="da1")
                    for m in range(2):
                        for k in range(2):  # feat chunks (4, 5)
                            nc.tensor.matmul(
                                da1[:, m, :],
                                sb["w_dec_a1"][:, 4 + k, m * 128 : (m + 1) * 128],
                                hs[:, t, k, :],
                                start=(m == 0 and k == 0),
                                stop=False,
                            )
                    for m in range(2):  # attn1 bias rows
                        nc.tensor.matmul(
                            da1[:, m, :],
                            sb["dba1_row"][0:1, m * 128 : (m + 1) * 128],
                            ones_b,
                            start=False,
                            stop=(m == 1),
                        )
                    nxt[t] = da1

                prefill(0)

                for t in range(dec_steps):
                    da1 = nxt.pop(t)
                    # attn2 bias row first (no deps; ps_s bufs=2 so no WAR
                    # stall at the PE queue head)
                    s_ps = ps_s.tile(
                        [128, TP], dt.float32, tag="s_ps", padded_shape=[128, 512]
                    )
                    nc.tensor.matmul(
                        s_ps, ones_row, sb["ba2_row"], start=True, stop=False
                    )

                    # attn1 h/c accumulate: c-parts FIRST (cd_b is ready ~1us
                    # before hdT -> these 4 MMs break up the tail PE-idle gap)
                    rhs_a1 = [
                        cd_b[:, 0, :],
                        cd_b[:, 1, :],
                        hdT[:, 0, :],
                        hdT[:, 1, :],
                    ]
                    wk_a1 = [2, 3, 0, 1]  # w_dec_a1 k-chunk for each rhs
                    for k in range(4):
                        for m in range(2):
                            nc.tensor.matmul(
                                da1[:, m, :],
                                sb["w_dec_a1"][:, wk_a1[k], m * 128 : (m + 1) * 128],
                                rhs_a1[k],
                                start=False,
                                stop=True,
                                skip_group_check=True,
                            )

                    # gates h-part: first 4 MMs cover the aTd-tanh latency,
                    # then attn2 jumps the queue, then the remaining 12
                    g8d = ps_dg.tile([128, 8, BL], dt.float32, tag="g8d", name="g8d")

                    def gate_h(j):
                        for k in range(2):
                            nc.tensor.matmul(
                                g8d[:, j, :],
                                sb["w_dec_g"][:, k, j * 128 : (j + 1) * 128],
                                hdT[:, k, :],
                                start=(j in (0, 4) and k == 0),
                                stop=(j in (3, 7) and k == 1),
                            )

                    for j in (0, 1):
                        gate_h(j)

                    # attn1 tanh (bias already in psum via prefill)
                    aTd = wp.tile([128, 2, BL], dt.bfloat16, tag="aTd")
                    nc.scalar.activation(out=aTd, in_=da1, func=AF.Tanh)

                    # attn2, batch-major: s[b, t']
                    for k in range(2):
                        nc.tensor.matmul(
                            s_ps,
                            aTd[:, k, :],
                            sb["w_dec_a2"][:, k, :],
                            start=False,
                            stop=(k == 1),
                        )

                    for j in (2, 3, 4, 5, 6, 7):
                        gate_h(j)

                    # prefill next step's attn1 feat part while the softmax
                    # chain runs on ACT/DVE, plus warmth fillers to keep the
                    # PE busy-density above the HAM un-throttle threshold
                    if t + 1 < dec_steps:
                        prefill(t + 1)
                    flt2 = ps_fl2.tile([128, 512], dt.float32, tag="fl2", name="fl2")
                    for fk in range(2):
                        nc.tensor.matmul(
                            flt2[:, 0:256],
                            sb["ident"],
                            sb["w_dec_g"][:, 0, (fk % 4) * 256 : (fk % 4) * 256 + 256],
                            start=True,
                            stop=True,
                        )

                    # softmax pieces: e, Z, dot_c, xi
                    e_sb = wp.tile([128, TP], dt.bfloat16, tag="e_sb")
                    z_t = wp.tile([128, 1], dt.float32, tag="z_t")
                    nc.scalar.activation(
                        out=e_sb, in_=s_ps, func=AF.Exp, accum_out=z_t
                    )
                    eq = wp.tile([128, TP], dt.bfloat16, tag="eq")
                    rz = wp.tile([128, 1], dt.float32, tag="rz")
                    dot_c = wp.tile([128, 1], dt.float32, tag="dot_c")
                    nc.vector.tensor_mul(eq, e_sb, qcT[:, 0:TP])
                    nc.vector.reciprocal(rz, z_t)
                    nc.vector.tensor_reduce(
                        out=dot_c, in_=eq, axis=mybir.AxisListType.X, op=OP.add
                    )
                    # xi = dot_c/Z + y*Wc00 (tgw precomputed on host)
                    xi_col = wp.tile([128, 1], dt.bfloat16, tag="xi_col")
                    nc.vector.scalar_tensor_tensor(
                        out=xi_col,
                        in0=dot_c,
                        scalar=rz,
                        in1=sb["tgw"][:, t : t + 1],
                        op0=OP.mult,
                        op1=OP.add,
                    )
                    # xi back to row layout: regular matmul against identity
                    # (out[0, n] = sum_k xi[k] * I[k, n] = xi[n])
                    xi_ps = ps_dx.tile([1, 128], dt.float32, tag="xi_ps")
                    nc.tensor.matmul(
                        xi_ps, xi_col, sb["ident"], start=True, stop=True
                    )
                    nc.vector.tensor_copy(xiT2[0:1, :], xi_ps[0:1, :])

                    # gates [xi; 1] chunk (bias folded): late accumulate
                    for j in range(8):
                        nc.tensor.matmul(
                            g8d[:, j, :],
                            sb["w_dec_gx"][:, j * 128 : (j + 1) * 128],
                            xiT2,
                            start=False,
                            stop=True,
                            skip_group_check=True,
                        )

                    # LSTM elementwise (tanh-form): tanh in two halves [i,f]
                    # then [g,o] so the DVE t2 op overlaps the second ACT
                    th8 = wp.tile([128, 8, BL], dt.float32, tag="th8")
                    nc.scalar.activation(
                        out=th8[:, 0:4, :], in_=g8d[:, 0:4, :], func=AF.Tanh, scale=0.5
                    )
                    nc.scalar.activation(
                        out=th8[:, 4:8, :], in_=g8d[:, 4:8, :], func=AF.Tanh, scale=0.5
                    )
                    # tail fillers anchored on th8 halves (fire mid-tail)
                    nc.tensor.matmul(
                        flt2[:, 0:128], sb["ident_f32"], th8[:, 0, :],
                        start=True, stop=True,
                    )
                    nc.tensor.matmul(
                        flt2[:, 0:128], sb["ident_f32"], th8[:, 4, :],
                        start=True, stop=True,
                    )
                    p1 = wp.tile([128, 2, BL], dt.float32, tag="dp1")
                    t2 = wp.tile([128, 2, BL], dt.float32, tag="dt2")
                    nc.vector.scalar_tensor_tensor(
                        out=t2,
                        in0=th8[:, 2:4, :],
                        scalar=1.0,
                        in1=cd_f,
                        op0=OP.add,
                        op1=OP.mult,
                    )
                    nc.vector.scalar_tensor_tensor(
                        out=p1,
                        in0=th8[:, 0:2, :],
                        scalar=1.0,
                        in1=th8[:, 4:6, :],
                        op0=OP.add,
                        op1=OP.mult,
                    )
                    # c'_new = 0.5 * t2 + p1
                    nc.vector.scalar_tensor_tensor(
                        out=cd_f,
                        in0=t2,
                        scalar=0.5,
                        in1=p1,
                        op0=OP.mult,
                        op1=OP.add,
                    )
                    nc.vector.tensor_copy(cd_b, cd_f)
                    # fillers anchored on cd_b + one free 512 behind them
                    nc.tensor.matmul(
                        flt2[:, 0:128], sb["ident"], cd_b[:, 0, :],
                        start=True, stop=True,
                    )
                    nc.tensor.matmul(
                        flt2[:, 0:128], sb["ident"], cd_b[:, 1, :],
                        start=True, stop=True,
                    )
                    nc.tensor.matmul(
                        flt2, sb["ident"], sb["w_dec_g"][:, 0, 0:512],
                        start=True, stop=True,
                    )
                    th_c = wp.tile([128, 2, BL], dt.float32, tag="th_c")
                    nc.scalar.activation(
                        out=th_c, in_=cd_f, func=AF.Tanh, scale=0.5
                    )
                    # h' = (th_o + 1) * th_c
                    nc.vector.scalar_tensor_tensor(
                        out=hdT,
                        in0=th8[:, 6:8, :],
                        scalar=1.0,
                        in1=th_c,
                        op0=OP.add,
                        op1=OP.mult,
                    )

                    if t == dec_steps - 1:
                        # dot_f ; out = hd.Wf + dot_f/Z + bf  (batch-major)
                        eq2 = wp.tile([128, TP], dt.bfloat16, tag="eq2")
                        dot_f = wp.tile([128, 1], dt.float32, tag="dot_f")
                        nc.vector.tensor_mul(eq2, e_sb, qfT[:, 0:TP])
                        nc.vector.tensor_reduce(
                            out=dot_f, in_=eq2, axis=mybir.AxisListType.X, op=OP.add
                        )
                        df = wp.tile([128, 1], dt.float32, tag="df")
                        nc.vector.tensor_mul(df, dot_f, rz)
                        fin_ps = flt2[:, 0:1]  # reuse the filler psum bank
                        for k in range(2):
                            nc.tensor.matmul(
                                fin_ps,
                                hdT[:, k, :],
                                sb["w_fh"][:, k, :],
                                start=(k == 0),
                                stop=(k == 1),
                            )
                        nc.vector.scalar_tensor_tensor(
                            out=o_col,
                            in0=df,
                            scalar=sb["consts_b"][:, 1:2],
                            in1=fin_ps,
                            op0=OP.add,
                            op1=OP.add,
                        )
                        nc.sync.dma_start(out=out_d.ap(), in_=o_col)

                # consume the filler psum so nothing upstream is elided
                fl_sb = sp.tile([128, 4], dt.float32, tag="fl_sb")
                nc.vector.tensor_copy(fl_sb, flt2[:, 0:4])
                nc.sync.dma_start(out=fl_d.ap(), in_=fl_sb)

    nc.finalize()
    return nc


def _get_nc():
    if "nc" not in _CACHE:
        _CACHE["nc"] = _build()
    return _CACHE["nc"]


def _run(inputs, **kw):
    from concourse.bass_utils import run_bass_kernel_spmd

    shared, per_core = _pack_inputs(inputs)
    nc = _get_nc()
    in_maps = []
    for c in range(NCORES):
        m = dict(shared)
        m.update(per_core[c])
        in_maps.append(m)
    res = run_bass_kernel_spmd(nc, in_maps, list(range(NCORES)), **kw)
    out = np.concatenate([np.asarray(res.results[c]["out"]) for c in range(NCORES)])
    return out.astype(np.float32).reshape(B, 1), res


def kernel(**inputs):
    return _run(inputs)[0]

